# revision 62
# baseline (speedup 1.0000x reference)
"""Trainium2 Bass kernel: negative-Jacobian-determinant penalty loss.

reference semantics:
    y = identity_grid + y_pred            # [B, D, H, W, 3]
    J = np.gradient-style central/one-sided diffs of y along (D, H, W)
    det = det3x3(J) per voxel; loss = mean(min(det, 0)^2)

Math used here:
  * gradient(identity_grid) == 1 exactly everywhere (incl. edges), so
    J = I + G with G[j][c] = grad_j(y_pred[c]).
  * one-sided edge diffs == central diffs over a linearly-extrapolated
    1-voxel pad, so the host pads and the device does only central diffs.
  * central diff = 0.5*(f[+1]-f[-1]); we compute raw diffs D = 2*G and
    det(I+G) = det(2I + D)/8, folding the /8 (squared: /64) into the
    final host-side scale.

Device layout (per core, shard = one (batch, D-quarter)):
  host-padded, host-f16-cast shard x[226, 3, 42, 194] in W-MAJOR
  (w, c, d, h) DRAM order: per partition (w) each channel's (d, h)
  block is contiguous, so every group load coalesces into >=512B DMA
  descriptors (below 512B the DMA engines pay a 2x latency penalty).
  partitions = W (two 128-row chunks), free dim = (d, h).
  All field tensors (a/b/c diffs, products) are stored COMPACT at
  Hw = 192 valid H columns; only the x reads use 4D windowed views of
  the Hp=194-wide padded input, so no pad columns flow through the
  elementwise pipeline.

Engine split (cost-model balanced; all four compute engines busy):
  * b = W-diffs via TensorE shift-matrix matmul into 2-bank PSUM tiles
    (2x 2-plane matmuls per tile), exited by one 3D-AP ScalarE
    activation per tile (b2's +2 diagonal offset folded into the exit
    bias).
  * a-diffs for ch0 AND ch1 (a_pe=2) ride TensorE +-I accumulated
    matmuls through the same PSUM ring (a0's +2 via its exit bias);
    only a2 plus the c-diffs run as DVE/GPSIMD subtracts, split at
    d-plane granularity with a carry so the aggregate ratio stays at
    skp (4D strided x views can't split mid-plane).
  * the m/x/t product passes are channel-BATCHED multi-channel 3D-AP
    instructions on the compact fields, each volume-split ~80/20
    between DVE (f16 2x mode) and GPSIMD.
  * the +2 offset on c1 is one tensor_scalar add per engine side
    (DVE 4x).
  * negdet = t0+t1+t2 is summed on TensorE via identity matmuls with
    relu fused into the ScalarE PSUM exit; Square+accumulate on
    ScalarE; ONE group (dve_tails=1, mid-stream) sums/relus on
    DVE/GPSIMD instead to relieve the ScalarE wall.
  * per-chunk W-validity masks + partition reduction via a masked
    Square and a GPSIMD XYZWC tensor_reduce; a tiny t~0 dummy
    activation pre-pulls the lazy 1283ns act-table load off the
    first real PSUM exit.
"""

import math
import os
import sys
from contextlib import ExitStack
from dataclasses import dataclass

import numpy as np

for _p in ("/root/.axon_site/_ro/trn_rl_repo", "/opt/trn_rl_repo"):
    if os.path.isdir(_p) and _p not in sys.path:
        sys.path.append(_p)

import concourse.bass as bass  # noqa: E402
import concourse.mybir as mybir  # noqa: E402
import concourse.tile as tile  # noqa: E402
from concourse import bacc  # noqa: E402
from concourse import bass_utils  # noqa: E402

F32 = mybir.dt.float32
F16 = mybir.dt.float16
Alu = mybir.AluOpType


@dataclass(frozen=True)
class Cfg:
    Dsh: int = 42   # shard D planes incl 1-plane halo each side
    Wp: int = 226   # padded W (1 halo each side)
    Hp: int = 194   # padded H (1 halo each side)
    kD: int = 8     # output D planes per group
    P: int = 128    # partition rows per W chunk
    skp: int = 800  # permille of each split pass on DVE; rest on GPSIMD
    skp_d: int = 0  # plane-granular diff split permille (0 = use skp)
    skp_f: int = 800  # first-group diff split permille
    warm: int = 0   # planes in the first chunk's warmup group (0 = off)
    cool: int = 2   # planes in the last chunk's cooldown group (0 = off)
    cool0: int = 0  # planes in a chunk-0 trailing cooldown group (0 = off)
    mode: str = "split"  # "split": every pass split skp/(1-skp); "xpool": X3 whole on GP, T3 split skp
    gp_stt: bool = False  # scalar_tensor_tensor on Pool crashes neuronxcc; keep False
    mm_sub: int = 2  # D planes per PE matmul sub-chunk (free <= 512 fp32 / bank)
    pair_exits: bool = True   # 2-bank PSUM tiles, one 3D-AP exit per 2 sub-chunks
    pe_s2: bool = True        # t0+t1+t2 summed on PE; relu fused into PSUM exit
    tslots: int = 2           # sub-chunk slots (banks) per t-sum PSUM tile/exit
    dve_tails: int = 0        # groups (evenly spread) whose t-sum+relu runs on
                              # DVE/Pool instead of PE+ScalarE (Act relief)
    a_pe: int = 1             # how many a-channels (from ch0) on PE via +-I matmuls
    defer_tail: bool = True   # software-pipeline the s2/relu/square tail one group behind
    interleave: bool = False  # round-robin groups across the two W-chunks
    plane_split: bool = False  # one shared DVE/Pool plane boundary per group
    xbufs: int = 2
    fbufs: int = 2
    mbufs: int = 2

    @property
    def w_chunks(self):
        """[(cw0, valid_lo, valid_hi)] local partition rows, inclusive."""
        chunks = []
        lo = 1  # first valid global w row
        last = self.Wp - 2
        while lo <= last:
            cw0 = min(lo - 1, self.Wp - self.P)
            hi = min(cw0 + self.P - 2, last)
            chunks.append((cw0, lo - cw0, hi - cw0))
            lo = hi + 1
        return chunks

    @property
    def d_groups(self):
        """[(d0, n_out)] group reads planes d0..d0+n_out+1, outputs d0+1..d0+n_out."""
        return self.d_groups_for(0)

    def d_groups_for(self, ci):
        """Per-chunk group sizes. A small warmup group may lead the first
        chunk, and a small cooldown group ends the last chunk so the
        drain tail (PE sum + exits + square) after the final DVE op is
        short."""
        last = self.Dsh - 2
        sizes = []
        rem = last
        warm = self.warm if ci == 0 and 0 < self.warm < self.kD else 0
        cool = self.cool if ci == 1 and 0 < self.cool < self.kD else (
            self.cool0 if ci == 0 and 0 < self.cool0 < self.kD else 0)
        if warm:
            sizes.append(warm)
            rem -= warm
        rem -= cool
        while rem > 0:
            n = min(self.kD, rem)
            sizes.append(n)
            rem -= n
        if cool:
            sizes.append(cool)
        groups = []
        d0 = 0
        for n in sizes:
            groups.append((d0, n))
            d0 += n
        return groups


def _consts(cfg: Cfg):
    """Host-side constant tensors: shift matrix + bias/mask columns."""
    P = cfg.P
    sc = np.zeros((P, P), dtype=np.float32)
    for m in range(P):
        if m + 1 < P:
            sc[m + 1, m] = 1.0
        if m - 1 >= 0:
            sc[m - 1, m] = -1.0
    chunks = cfg.w_chunks
    bm = np.zeros((P, 2 + len(chunks)), dtype=np.float32)
    bm[:, 1] = 2.0
    for ci, (_, lo, hi) in enumerate(chunks):
        bm[lo : hi + 1, 2 + ci] = 1.0
    idm = np.eye(P, dtype=np.float16)
    idn = (-np.eye(P)).astype(np.float16)
    return {"sc": sc.astype(np.float16), "bm": bm, "idm": idm, "idn": idn}


def build_nc(cfg: Cfg):
    nc = bacc.Bacc("TRN2", target_bir_lowering=False, debug=False)
    P, Hp = cfg.P, cfg.Hp
    chunks = cfg.w_chunks
    groups = cfg.d_groups
    chunk_groups = [cfg.d_groups_for(ci) for ci in range(len(chunks))]
    n_slots = sum(len(g) for g in chunk_groups)

    # w-major DRAM layout: per partition (w) each channel's (d, h) block is
    # contiguous, so group loads coalesce into >=512B descriptors (the DMA
    # engines pay a 2x latency multiplier below 512B)
    x_d = nc.dram_tensor("x", [cfg.Wp, 3, cfg.Dsh, Hp], F16, kind="ExternalInput").ap()
    sc_d = nc.dram_tensor("sc", [P, P], F16, kind="ExternalInput").ap()
    idm_d = nc.dram_tensor("idm", [P, P], F16, kind="ExternalInput").ap()
    idn_d = nc.dram_tensor("idn", [P, P], F16, kind="ExternalInput").ap()
    bm_d = nc.dram_tensor("bm", [P, 2 + len(chunks)], F32, kind="ExternalInput").ap()
    out_d = nc.dram_tensor("out", [1, 1], F32, kind="ExternalOutput").ap()

    with tile.TileContext(nc) as tc, ExitStack() as ctx:
        cpool = ctx.enter_context(tc.tile_pool(name="consts", bufs=1))
        xpool = ctx.enter_context(tc.tile_pool(name="x", bufs=cfg.xbufs))
        fpool = ctx.enter_context(tc.tile_pool(name="fields", bufs=cfg.fbufs))
        bpool = ctx.enter_context(
            tc.tile_pool(name="bfield", bufs=int(os.environ.get("DETK_BB", "2")))
        )
        mpool = ctx.enter_context(tc.tile_pool(name="mags", bufs=cfg.mbufs))
        apool = ctx.enter_context(tc.tile_pool(name="acc", bufs=1))
        pp = ctx.enter_context(
            tc.tile_pool(name="psum", bufs=int(os.environ.get("DETK_PPB", "2")), space="PSUM")
        )
        pt = ctx.enter_context(
            tc.tile_pool(name="tpsum", bufs=int(os.environ.get("DETK_PTB", "2")), space="PSUM")
        )

        # consts are loaded AFTER the first group's x pieces (they are
        # only needed by PE/ScalarE, which start later than the DVE diffs)
        sc_sb = cpool.tile([P, P], F16)
        idm_sb = cpool.tile([P, P], F16, tag="idm", name="idm")
        idn_sb = cpool.tile([P, P], F16, tag="idn", name="idn")
        bm_sb = cpool.tile([P, 2 + len(chunks)], F32)
        zvec = bm_sb[:, 0:1]

        def load_consts():
            nc.scalar.dma_start(idm_sb[:], idm_d)
            nc.scalar.dma_start(idn_sb[:], idn_d)

        # sc via the (idle) ScalarE DGE queue and bm first on SP, so PE's
        # b-matmuls and the exits (bias reads bm) start as soon as the
        # first x piece lands -- without delaying the x pieces behind
        # const DMAs on the SP queue
        nc.scalar.dma_start(sc_sb[:], sc_d)
        nc.sync.dma_start(bm_sb[:], bm_d)

        acc = apool.tile([P, n_slots], F32)

        # tiny dummy activations: pull the lazy 1283ns act-table load (and
        # the Act sequencer start) to t~0 instead of blocking the first
        # real PSUM exit
        warmup = cpool.tile([P, 2], F16, tag="actwarm", name="actwarm")
        nc.scalar.activation(
            warmup[:, 0:1], acc[:, 0:1],
            mybir.ActivationFunctionType.Square, bias=0.0, scale=1.0,
        )
        nc.scalar.activation(
            warmup[:, 1:2], acc[:, 0:1],
            mybir.ActivationFunctionType.Relu, bias=0.0, scale=1.0,
        )

        # chosen evenly over execution order; filled after seq is built
        dve_tail_slots = set()

        pending_tail = None
        ngc_limit = int(os.environ.get("DETK_NGC", "0"))
        ngc_done = 0
        slot_off = [0]
        for g in chunk_groups[:-1]:
            slot_off.append(slot_off[-1] + len(g))
        seq = [
            (ci, gi, d0, nD)
            for ci in range(len(chunks))
            for gi, (d0, nD) in enumerate(chunk_groups[ci])
        ]
        if cfg.interleave:
            from itertools import zip_longest
            g0 = [s for s in seq if s[0] == 0]
            g1 = [s for s in seq if s[0] == 1]
            seq = []
            for a, b in zip_longest(g0, g1):
                if a is not None:
                    seq.append(a)
                if b is not None:
                    seq.append(b)
        dt_idx = os.environ.get("DETK_DTIDX", "2" if cfg.dve_tails == 1 else "")
        if dt_idx and cfg.dve_tails:
            for tok in dt_idx.split(","):
                sci, sgi, _, _ = seq[int(tok)]
                dve_tail_slots.add(slot_off[sci] + sgi)
        else:
            for i in range(cfg.dve_tails):
                sci, sgi, _, _ = seq[int((i + 0.5) * len(seq) / cfg.dve_tails)]
                dve_tail_slots.add(slot_off[sci] + sgi)
        dk_carry = [0.0]
        for seq_i, (ci, gi, d0, nD) in enumerate(seq):
            cw0 = chunks[ci][0]
            if True:
                if ngc_limit and ngc_done >= ngc_limit:
                    continue
                ngc_done += 1
                KD2 = nD + 2
                Hw = Hp - 2  # valid H window; fields stored compact at Hw
                Fx = KD2 * Hp
                Ff = nD * Hw
                den = 1000 if cfg.skp > 100 else 100
                if cfg.plane_split:
                    # one plane boundary per group shared by ALL stages:
                    # DVE owns planes [0:dk), Pool [dk:nD) — the two
                    # engines' stripes never read each other's writes
                    ideal = nD * cfg.skp / den + dk_carry[0]
                    dk = max(0, min(nD, int(round(ideal))))
                    dk_carry[0] = ideal - dk
                    Fd = dk * Hw
                else:
                    dk = None
                    Fd = (Ff * cfg.skp // den) & ~1  # DVE part [0:Fd)
                Fg = Ff - Fd

                xt = xpool.tile([P, 3, Fx], F16, tag="x", name="x")
                first_gc = seq_i == 0
                if first_gc:
                    # per-channel pieces so the first diffs start after a
                    # small fraction of the load
                    pb = [
                        int(t) for t in os.environ.get(
                            "DETK_PIECES", "5").split(",") if t
                    ]
                    pb = [b for b in pb if 0 < b < KD2]
                    edges = [0] + pb + [KD2]
                    pieces = tuple(
                        (lo, hi) for lo, hi in zip(edges, edges[1:]) if hi > lo
                    )
                    # ch0/ch2 pieces on the SP queue; ch1 pieces ride the
                    # ScalarE DGE queue (idle at t~0, already carrying sc)
                    # so the two queues issue the first load in parallel
                    ch1q = (
                        nc.scalar
                        if os.environ.get("DETK_CH1Q", "0") == "1"
                        else nc.sync
                    )
                    for ch in (0, 1, 2):
                        src = x_d[cw0 : cw0 + P, ch, d0 : d0 + KD2, :]
                        dst = xt[:, ch].rearrange("p (d h) -> p d h", d=KD2)
                        q = ch1q if ch == 1 else nc.sync
                        for lo_d, hi_d in pieces:
                            q.dma_start(
                                dst[:, lo_d:hi_d].rearrange("p d h -> p (d h)"),
                                src[:, lo_d:hi_d].rearrange("p d h -> p (d h)"),
                            )
                        if ch == 1:
                            load_consts()  # idm/idn after ch1 on the Act queue
                else:
                    # one DMA for all 3 channels; per partition each channel
                    # is one contiguous (d h) run
                    src = x_d[cw0 : cw0 + P, :, d0 : d0 + KD2, :].rearrange(
                        "p c d h -> p c (d h)"
                    )
                    nc.sync.dma_start(xt[:], src)

                def split_op(out, u, v, op, fd=None, base=None, width=None):
                    """Channel-batched split op on [P, CH, W]-style views:
                    out = u op v, DVE on innermost [0:fd), GPSIMD on [fd:W)
                    where W is the view's innermost width."""
                    fd = Fd if fd is None else fd
                    w = out.shape[-1] if width is None else width
                    fd = min(fd, w)
                    dve = {
                        Alu.subtract: nc.vector.tensor_sub,
                        Alu.mult: nc.vector.tensor_mul,
                        Alu.add: nc.vector.tensor_add,
                    }[op]
                    gp = {
                        Alu.subtract: nc.gpsimd.tensor_sub,
                        Alu.mult: nc.gpsimd.tensor_mul,
                        Alu.add: nc.gpsimd.tensor_add,
                    }[op]
                    if fd > 0:
                        dve(out[:, :, 0:fd], u[:, :, 0:fd], v[:, :, 0:fd])
                    if fd < w:
                        gp(out[:, :, fd:w], u[:, :, fd:w], v[:, :, fd:w])

                # 4D windowed view of x: [P, c, d(KD2), h(Hp)]
                x4 = xt[:].rearrange("p c (d h) -> p c d h", d=KD2)

                pc_carry = [0.0]

                def dsplit_op(out4, u4, v4, op, planes, p0=0):
                    """Plane-granular split op on [P, CH, DP, Hw] views (4D
                    strided inputs can't split mid-plane): DVE gets the
                    first dk planes of every channel, GPSIMD the rest; a
                    running carry keeps the aggregate ratio at skp. The
                    first group splits Pool-heavy so GPSIMD ramps up
                    instead of idling behind DVE."""
                    CH = out4.shape[1]
                    if cfg.plane_split and dk is not None:
                        dkp = max(0, min(planes, dk - p0))
                    else:
                        skp_d = cfg.skp_f if first_gc else (cfg.skp_d or cfg.skp)
                        ideal = planes * CH * skp_d / den + pc_carry[0]
                        dkp = max(0, min(planes, int(round(ideal / CH))))
                        pc_carry[0] = ideal - dkp * CH
                    dve = {
                        Alu.subtract: nc.vector.tensor_sub,
                        Alu.mult: nc.vector.tensor_mul,
                        Alu.add: nc.vector.tensor_add,
                    }[op]
                    gp = {
                        Alu.subtract: nc.gpsimd.tensor_sub,
                        Alu.mult: nc.gpsimd.tensor_mul,
                        Alu.add: nc.gpsimd.tensor_add,
                    }[op]
                    if dkp > 0:
                        dve(out4[:, :, 0:dkp], u4[:, :, 0:dkp], v4[:, :, 0:dkp])
                    if dkp < planes:
                        gp(out4[:, :, dkp:planes], u4[:, :, dkp:planes],
                           v4[:, :, dkp:planes])

                # --- b = W-diffs on TensorE + ScalarE exits --------------
                # emitted first so PE/ScalarE start as soon as x lands
                B = bpool.tile([P, 3, Ff], F16, tag="B", name="B")
                sub = cfg.mm_sub
                if cfg.pair_exits and nD % sub == 0:
                    # first group: channel-major order so each channel's B
                    # completes as soon as that channel's DMA lands
                    p0s = list(range(0, nD, 2 * sub))
                    iters = (
                        [(p0, ch) for ch in range(3) for p0 in p0s]
                        if first_gc
                        else [(p0, ch) for p0 in p0s for ch in range(3)]
                    )
                    for p0, ch in iters:
                        nh = 2 if p0 + 2 * sub <= nD else 1
                        if True:
                            bp = pp.tile([P, 1024], F32, tag="bpsum", name="bpsum")
                            for half in range(nh):
                                q0 = p0 + half * sub
                                rhs = x4[:, ch, 1 + q0 : 1 + q0 + sub, 1 : 1 + Hw]
                                nc.tensor.matmul(
                                    bp[:, half * 512 : half * 512 + sub * Hw],
                                    sc_sb[:], rhs, start=True, stop=True,
                                )
                            srcv = bp[:].rearrange("p (k x) -> p k x", k=2)[
                                :, 0:nh, 0 : sub * Hw
                            ]
                            dstv = B[:, ch][
                                :, p0 * Hw : (p0 + nh * sub) * Hw
                            ].rearrange("p (k x) -> p k x", k=nh)
                            nc.scalar.activation(
                                dstv, srcv, mybir.ActivationFunctionType.Identity,
                                bias=bm_sb[:, 1:2] if ch == 2 else zvec, scale=1.0,
                            )
                else:
                    for p0 in range(0, nD, sub):
                        pn = min(sub, nD - p0)
                        for ch in range(3):
                            bp = pp.tile([P, pn * Hw], F32, tag="bpsum", name="bpsum")
                            rhs = x4[:, ch, 1 + p0 : 1 + p0 + pn, 1 : 1 + Hw]
                            nc.tensor.matmul(bp[:], sc_sb[:], rhs, start=True, stop=True)
                            dst = B[:, ch][:, p0 * Hw : (p0 + pn) * Hw]
                            if ch == 2:
                                nc.scalar.activation(
                                    dst, bp[:], mybir.ActivationFunctionType.Identity,
                                    bias=bm_sb[:, 1:2], scale=1.0,
                                )
                            else:
                                nc.scalar.copy(dst, bp[:])

                # --- gradient fields (channel-batched) -------------------
                # one tile holds (a0,a1,a2,c0,c1,c2) so the two +2 diagonal
                # offsets (a0 at ch0, c1 at ch4) are one stride-4 TS op
                xp = cfg.mode == "xpool"
                fd_main = Ff if xp else Fd   # diffs/m-ops: all-DVE in xpool mode
                AC = fpool.tile([P, 6, Ff], F16, tag="AC", name="AC")
                A = AC[:, 0:3]
                C = AC[:, 3:6]
                n_ape = cfg.a_pe if nD % sub == 0 else 0
                for ach in range(n_ape):
                    # a[ach] = x[ach][d+2] - x[ach][d] (+2 for ch0) on
                    # TensorE: +-I accumulated matmuls through the bpsum
                    # ring, the +2 via the exit bias
                    for p0 in range(0, nD, 2 * sub):
                        nh = 2 if p0 + 2 * sub <= nD else 1
                        ap4 = pp.tile([P, 1024], F32, tag="bpsum", name="bpsum")
                        for half in range(nh):
                            q0 = p0 + half * sub
                            dstp = ap4[:, half * 512 : half * 512 + sub * Hw]
                            # two single-matmul groups accumulating via
                            # start=False: a weights switch INSIDE one
                            # accumulation group is fatal on hardware
                            # (NRT_EXEC_UNIT_UNRECOVERABLE), between groups
                            # it is the same pattern as the b-exits
                            nc.tensor.matmul(
                                dstp, idm_sb[:],
                                x4[:, ach, q0 + 2 : q0 + 2 + sub, 1 : 1 + Hw],
                                start=True, stop=True,
                            )
                            nc.tensor.matmul(
                                dstp, idn_sb[:],
                                x4[:, ach, q0 : q0 + sub, 1 : 1 + Hw],
                                start=False, stop=True, skip_group_check=True,
                            )
                        srcv = ap4[:].rearrange("p (k x) -> p k x", k=2)[
                            :, 0:nh, 0 : sub * Hw
                        ]
                        dstv = A[:, ach][
                            :, p0 * Hw : (p0 + nh * sub) * Hw
                        ].rearrange("p (k x) -> p k x", k=nh)
                        nc.scalar.activation(
                            dstv, srcv, mybir.ActivationFunctionType.Identity,
                            bias=bm_sb[:, 1:2] if ach == 0 else zvec, scale=1.0,
                        )
                a_chs = tuple(range(n_ape, 3))
                A4 = A.rearrange("p c (d h) -> p c d h", d=nD)
                C4 = C.rearrange("p c (d h) -> p c d h", d=nD)
                if first_gc:
                    # first group: per-channel, per-DMA-piece diffs so DVE/GP
                    # start as soon as each piece of the load lands
                    dedges = [0] + [min(max(b - 2, 0), nD) for b in pb] + [nD]
                    dedges = sorted(set(dedges))
                    dpieces = list(zip(dedges, dedges[1:]))
                    for ch in range(3):
                        for plo, phi in dpieces:
                            if ch in a_chs:
                                dsplit_op(
                                    A4[:, ch : ch + 1, plo:phi],
                                    x4[:, ch : ch + 1, plo + 2 : phi + 2, 1 : 1 + Hw],
                                    x4[:, ch : ch + 1, plo:phi, 1 : 1 + Hw],
                                    Alu.subtract, phi - plo, p0=plo,
                                )
                            dsplit_op(
                                C4[:, ch : ch + 1, plo:phi],
                                x4[:, ch : ch + 1, plo + 1 : phi + 1, 2 : 2 + Hw],
                                x4[:, ch : ch + 1, plo + 1 : phi + 1, 0:Hw],
                                Alu.subtract, phi - plo, p0=plo,
                            )
                else:
                    if n_ape < 3:
                        dsplit_op(
                            A4[:, n_ape:3],
                            x4[:, n_ape:3, 2 : 2 + nD, 1 : 1 + Hw],
                            x4[:, n_ape:3, 0:nD, 1 : 1 + Hw],
                            Alu.subtract, nD,
                        )
                    dsplit_op(
                        C4,
                        x4[:, :, 1 : 1 + nD, 2 : 2 + Hw],
                        x4[:, :, 1 : 1 + nD, 0:Hw],
                        Alu.subtract, nD,
                    )
                # +2 offsets on c1 (and a0 when not PE-exited; b2 rides its
                # exit bias)
                pm = AC[:, 4:5] if n_ape else AC[:, 0::4]
                if fd_main > 0:
                    nc.vector.tensor_scalar_add(
                        pm[:, :, 0:fd_main], pm[:, :, 0:fd_main], 2.0
                    )
                if fd_main < Ff:
                    nc.gpsimd.tensor_scalar_add(
                        pm[:, :, fd_main:Ff], pm[:, :, fd_main:Ff], 2.0
                    )

                # --- det stage (channel-batched) -------------------------
                # M6 = (m1,m2, m3,m4, m5,m6); pairs chosen so the three
                # cross-terms fall out of one stride-2 batched subtract:
                #   x0 = m1-m2 = c2*b1 - c1*b2
                #   x1 = m3-m4 = c0*b2 - c2*b0
                #   x2 = m5-m6 = c1*b0 - c0*b1
                M6 = mpool.tile([P, 6, Ff], F16, tag="M6", name="M6")
                m_ops = [
                    (M6[:, 0:2], C[:, 2:0:-1], B[:, 1:3]),   # m1,m2
                    (M6[:, 2:4], C[:, 0::2], B[:, 2::-2]),   # m3,m4
                    (M6[:, 4:6], C[:, 1::-1], B[:, 0:2]),    # m5,m6
                ]
                if first_gc:
                    m_ops = m_ops[::-1]  # earliest-ready B channels first
                for dst_m, u_m, v_m in m_ops:
                    split_op(dst_m, u_m, v_m, Alu.mult, fd=fd_main)

                X3 = mpool.tile([P, 3, Ff], F16, tag="X3", name="X3")
                split_op(X3, M6[:, 0::2], M6[:, 1::2], Alu.subtract,
                         fd=0 if xp else Fd)

                T3 = mpool.tile([P, 3, Ff], F16, tag="T3", name="T3")
                split_op(T3, A, X3, Alu.mult)

                slot = slot_off[ci] + gi

                def tail(T3=T3, nD=nD, Ff=Ff, Fd=Fd, Fg=Fg, slot=slot,
                         ci=ci, split_op=split_op, is_last=False):
                    if (cfg.pe_s2 and nD % sub == 0 and not is_last
                            and slot not in dve_tail_slots):
                        # negdet = t0+t1+t2 accumulated on TensorE via
                        # identity matmuls into multi-bank PSUM tiles (one
                        # 512-fp32 bank slot per sub-chunk); relu fused into
                        # one PSUM exit per tile.
                        r = mpool.tile([P, Ff], F16, tag="r", name="r")
                        tsl = cfg.tslots
                        for p0 in range(0, nD, tsl * sub):
                            nh = min(tsl, (nD - p0) // sub)
                            tp = pt.tile(
                                [P, 512 * tsl], F32, tag="tpsum", name="tpsum"
                            )
                            for si in range(nh):
                                q0 = p0 + si * sub
                                dstp = tp[:, si * 512 : si * 512 + sub * Hp]
                                for ti in range(3):
                                    nc.tensor.matmul(
                                        dstp, idm_sb[:],
                                        T3[:, ti][:, q0 * Hp : (q0 + sub) * Hp],
                                        start=(ti == 0), stop=(ti == 2),
                                    )
                            srcv = tp[:].rearrange("p (k x) -> p k x", k=tsl)[
                                :, 0:nh, 0 : sub * Hp
                            ]
                            dstv = r[
                                :, p0 * Hp : (p0 + nh * sub) * Hp
                            ].rearrange("p (k x) -> p k x", k=nh)
                            nc.scalar.activation(
                                dstv, srcv, mybir.ActivationFunctionType.Relu,
                                bias=zvec, scale=1.0,
                            )
                    else:
                        s = mpool.tile([P, Ff], F16, tag="s", name="s")
                        split_op(
                            s[:].unsqueeze(1),
                            T3[:, 0:1], T3[:, 1:2], Alu.add,
                        )
                        s2 = mpool.tile([P, Ff], F16, tag="s2", name="s2")
                        split_op(
                            s2[:].unsqueeze(1),
                            s[:].unsqueeze(1), T3[:, 2:3], Alu.add,
                        )
                        r = mpool.tile([P, Ff], F16, tag="r", name="r")
                        if Fd > 0:
                            nc.vector.tensor_scalar_max(r[:, 0:Fd], s2[:, 0:Fd], 0.0)
                        if Fg > 0:
                            nc.gpsimd.tensor_scalar_max(r[:, Fd:Ff], s2[:, Fd:Ff], 0.0)

                    junk = mpool.tile([P, nD * (Hp - 2)], F16, tag="junk", name="junk")
                    rw = r[:].rearrange("p (d h) -> p d h", d=nD)[:, :, 1 : Hp - 1]
                    jw = junk[:].rearrange("p (d h) -> p d h", d=nD)
                    # scale = this chunk's 0/1 W-validity mask: (mask*r)^2
                    # == mask*r^2, so the accumulated sums come out masked
                    nc.scalar.activation(
                        jw, rw, mybir.ActivationFunctionType.Square,
                        bias=zvec, scale=bm_sb[:, 2 + ci : 3 + ci],
                        accum_out=acc[:, slot : slot + 1],
                    )

                if cfg.defer_tail:
                    if pending_tail is not None:
                        pending_tail()
                    pending_tail = tail
                else:
                    tail()

        if pending_tail is not None:
            pending_tail(is_last=True)

        # slots are already chunk-masked (Square's scale); one X-reduce,
        # one GPSIMD partition reduce, DMA out
        accC = apool.tile([P, 1], F32, tag="accC", name="accC")
        nc.vector.tensor_reduce(
            accC[:], acc[:], axis=mybir.AxisListType.X, op=mybir.AluOpType.add
        )
        outs = apool.tile([1, 1], F32, tag="outs", name="outs")
        nc.gpsimd.tensor_reduce(
            outs[:], accC[:], axis=mybir.AxisListType.XYZWC, op=mybir.AluOpType.add
        )
        nc.sync.dma_start(out_d, outs[:])

    nc.compile()
    return nc


# ----------------------------------------------------------------------------
# host-side data prep
# ----------------------------------------------------------------------------

def _pad_extrap(x, axis, n_lo, n_hi):
    """Pad with linear extrapolation of the edge (1 real pad plane), then
    replicate it for any extra (never-consumed, finiteness-only) planes."""
    def take(i):
        sl = [slice(None)] * x.ndim
        sl[axis] = slice(i, i + 1) if i >= 0 else slice(i, None if i == -1 else i + 1)
        return x[tuple(sl)]

    lo = 2.0 * take(0) - take(1)
    hi = 2.0 * take(-1) - take(-2)
    parts = [lo] * n_lo + [x] + [hi] * n_hi
    return np.concatenate(parts, axis=axis)


def prepare_shards(y_pred: np.ndarray, cfg: Cfg):
    """[B, 3, D, H, W] fp32 -> list of 8 f16 shards [Wp, 3, Dsh, Hp] (w,c,d,h)."""
    B, C, D, H, W = y_pred.shape
    x = np.asarray(y_pred, dtype=np.float32)
    x = _pad_extrap(x, 2, 1, 1)          # D -> D+2
    x = _pad_extrap(x, 3, 1, 1)          # H -> H+2
    x = _pad_extrap(x, 4, 1, 1)          # W -> W+2
    # (B, W, C, D, H): w-major so each partition's (c, d, h) is contiguous
    x = np.ascontiguousarray(x.transpose(0, 4, 1, 2, 3)).astype(np.float16)
    nq = 8 // B
    dq = D // nq
    shards = []
    for b in range(B):
        for q in range(nq):
            shards.append(np.ascontiguousarray(x[b, :, :, dq * q : dq * q + dq + 2]))
    return shards


def shard_ref_sum(xs: np.ndarray, cfg: Cfg) -> float:
    """Numpy mirror of the device computation for one shard (for testing)."""
    x = xs.astype(np.float64).transpose(1, 2, 0, 3)  # (w,c,d,h) -> (c,d,w,h)
    _, Dsh, Wp, Hp = x.shape
    dd, ww, hh = slice(1, Dsh - 1), slice(1, Wp - 1), slice(1, Hp - 1)
    a = x[:, 2:, ww, hh] - x[:, : Dsh - 2, ww, hh]
    c = x[:, dd, ww, 2:Hp] - x[:, dd, ww, 0 : Hp - 2]
    b = x[:, dd, 2:, hh] - x[:, dd, : Wp - 2, hh]
    a[0] += 2.0
    c[1] += 2.0
    b[2] += 2.0
    det = (
        a[0] * (c[1] * b[2] - c[2] * b[1])
        - a[1] * (c[0] * b[2] - c[2] * b[0])
        + a[2] * (c[0] * b[1] - c[1] * b[0])
    )
    neg = np.maximum(-det, 0.0)
    return float(np.sum(neg * neg))


# ----------------------------------------------------------------------------
# entry point
# ----------------------------------------------------------------------------

_CACHE: dict = {}


def _get_nc(cfg: Cfg):
    if cfg not in _CACHE:
        _CACHE[cfg] = build_nc(cfg)
    return _CACHE[cfg]


def run_shards(shards, cfg: Cfg, trace=False):
    nc = _get_nc(cfg)
    consts = _consts(cfg)
    in_maps = [{"x": s, **consts} for s in shards]
    res = bass_utils.run_bass_kernel_spmd(
        nc, in_maps, core_ids=list(range(len(shards))), trace=trace
    )
    sums = [float(r["out"][0, 0]) for r in res.results]
    return sums, res


def _cfg_from_env(D=160, H=192, W=224, B=2) -> Cfg:
    nq = 8 // B
    return Cfg(
        Dsh=D // nq + 2,
        Wp=W + 2,
        Hp=H + 2,
        kD=int(os.environ.get("DETK_KD", "8")),
        skp=int(os.environ.get("DETK_SKP", "800")),
        skp_d=int(os.environ.get("DETK_SKPD", "0")),
        skp_f=int(os.environ.get("DETK_SKPF", "800")),
        plane_split=os.environ.get("DETK_PSPLIT", "0") == "1",
        mode=os.environ.get("DETK_MODE", "split"),
        a_pe=int(os.environ.get("DETK_APE", "2")),
        dve_tails=int(os.environ.get("DETK_DT", "1")),
        warm=int(os.environ.get("DETK_WARM", "0")),
        cool=int(os.environ.get("DETK_COOL", "4")),
        cool0=int(os.environ.get("DETK_COOL0", "0")),
        defer_tail=os.environ.get("DETK_DEFER", "1") == "1",
        gp_stt=os.environ.get("DETK_GPSTT", "0") == "1",
        pair_exits=os.environ.get("DETK_PAIR", "1") == "1",
        pe_s2=os.environ.get("DETK_PES2", "1") == "1",
        xbufs=int(os.environ.get("DETK_XB", "2")),
        fbufs=int(os.environ.get("DETK_FB", "3")),
        mbufs=int(os.environ.get("DETK_MB", "2")),
    )


def kernel(y_pred: np.ndarray) -> np.ndarray:
    B, C, D, H, W = y_pred.shape
    cfg = _cfg_from_env(D, H, W, B)
    shards = prepare_shards(y_pred, cfg)
    # The device can transiently wedge (NRT_EXEC_UNIT_UNRECOVERABLE recovers
    # on re-run); retry, then fall back to the conservative config without
    # the PE a-diff accumulation.
    import dataclasses
    attempts = [cfg, cfg, dataclasses.replace(cfg, a_pe=0)]
    last_err = None
    for acfg in attempts:
        try:
            sums, _ = run_shards(shards, acfg)
            break
        except Exception as e:  # noqa: BLE001
            last_err = e
    else:
        raise last_err
    total = math.fsum(sums)
    mean = total / 64.0 / float(B * D * H * W)
    return np.array(mean, dtype=np.float32)


if __name__ == "__main__":
    np.random.seed(0)
    yp = np.random.randn(2, 3, 160, 192, 224).astype(np.float32)
    print(kernel(yp))

